# revision 8
# baseline (speedup 1.0000x reference)
"""GCN (2-layer, Citeseer-style) on 8 Trainium2 NeuronCores.

v2: layer-2 aggregation via src-local messages + SBUF-dst dma_scatter_add
+ ReduceScatter, replacing the AG2 + per-edge dma_gather path (scatter
descriptors carry 24B -> 7ns floor vs gather's 256B -> 22.8ns).

  - Phase 1 (unchanged): support = x @ W1, fp8 x / fp8 W1 (x64) DoubleRow
    matmuls; support bf16 (scaled), AllGathered in A/B halves with the
    A-half AG overlapping phase 1.
  - Phase 3 (unchanged): L1 aggregation by dst tile via batched dma_gather
    + selection-matrix matmuls; bias + leaky_relu -> h1T bf16.
  - Phase 5/6 (new, fused into the tile loop): s2_t = h1_t @ W2 -> [128,6]
    bf16. Per (tile, dst-half) chunk of <=128 out-edges: sel[e,s] =
    w_e*(src_e==s) on DVE, PE transpose -> selT, matmul(lhsT=selT,
    rhs=s2_t) -> per-edge messages [128,6] f32 -> ring; every 8 chunks a
    dma_scatter_add accumulates into SBUF accumulator pairs (parity-split,
    one pair per dst-half). Host scheduling guarantees scatter tokens are
    unique within each call (hw RMW races drop duplicate updates);
    conflicting edges (~2%) are ejected to a spill stream processed at the
    end via dma_gather from a local s2 DRAM table + per-lane w multiply,
    scattered in duplicate-free single-chunk calls.
  - Accumulators (all 50176 dst rows as (half,parity,p,group)) are dumped
    to DRAM, combined by ReduceScatter; each core receives exactly its own
    6272 rows in local-row order; + b2, log_softmax, output.
"""
import sys

sys.path.insert(0, "/opt/trn_rl_repo")

import numpy as np

from concourse import bass, bacc, mybir, tile
from concourse.bass_utils import run_bass_kernel_spmd

F32 = mybir.dt.float32
BF16 = mybir.dt.bfloat16
FP8 = mybir.dt.float8e4
I16 = mybir.dt.int16
X_FP8 = True
DR = True
W1_SCALE = 64.0

N_NODES = 50000
N_EDGES = 400000
F_IN = 3703
F_HID = 256
F_OUT = 6

CORES = 8
P = 128
TILES = 49
NPC = TILES * P            # 6272
NTOT = CORES * NPC         # 50176
TILES_A = 28
NPCA = TILES_A * P         # 3584
TILES_B = TILES - TILES_A
NPCB = NPC - NPCA          # 2688
HALF_A = CORES * NPCA      # 28672
HALF_B = CORES * NPCB      # 21504
KT = 30 if DR else 29
KPAD = KT * P
NB = 512
GB1 = 8                    # chunks per batched dma_gather call
SC_B = 32                  # chunks per primary scatter call
SEG = 8                    # transpose-segment chunks (one PSUM bank)
SP_CAP = 8                 # max chunks per spill call (gather max 1024)
GHALF = 98                 # scatter groups per half; group 98 = dump
NGRP = GHALF + 1
DUMP0 = 2 * GHALF * P      # first dump token (slot 196, p 0)

LAST_RESULT = None
_CACHE = {}


def _build(CL, CU, sched, num_devices=CORES, with_ag=True):
    CH = CL + CU
    idx_cols = TILES * CH * 8
    chunk_tile = sched["chunk_tile"]   # per global L2 chunk
    chunk_h = sched["chunk_h"]
    chunk_seq = sched["chunk_seq"]     # per-h sequence number
    call_bounds = sched["call_bounds"]  # {h: [(lo, hi), ...]}
    nch_h = sched["nch_h"]             # {h: count}
    sp_calls = sched["sp_calls"]       # [(lo, hi, h, maxtile)]
    spc = sched["spc"]
    nch2 = len(chunk_tile)
    sc_cols = (nch_h[0] + nch_h[1]) * 8

    nc = bacc.Bacc("TRN2", target_bir_lowering=False, debug=False,
                   num_devices=num_devices)

    XDT = FP8 if X_FP8 else BF16
    W1DT = FP8 if DR else BF16
    xT = nc.dram_tensor("xT", [KPAD, NPC], XDT, kind="ExternalInput")
    W1p = nc.dram_tensor("W1p", [P, KT * F_HID], W1DT, kind="ExternalInput")
    W2p = nc.dram_tensor("W2p", [F_HID, F_OUT], BF16, kind="ExternalInput")
    b1t = nc.dram_tensor("b1t", [P, 2], F32, kind="ExternalInput")
    b2b = nc.dram_tensor("b2b", [P, F_OUT], F32, kind="ExternalInput")
    idxd = nc.dram_tensor("idxd", [P, idx_cols], I16, kind="ExternalInput")
    dstd = nc.dram_tensor("dstd", [P, TILES * CH], F32, kind="ExternalInput")
    wd = nc.dram_tensor("wd", [P, TILES * CH], F32, kind="ExternalInput")
    seld = nc.dram_tensor("seld", [P, max(nch2, 1)], F32,
                          kind="ExternalInput")
    selwd = nc.dram_tensor("selwd", [P, max(nch2, 1)], F32,
                           kind="ExternalInput")
    scidxd = nc.dram_tensor("scidxd", [P, max(sc_cols, 8)], I16,
                            kind="ExternalInput")
    spgid = nc.dram_tensor("spgid", [P, max(spc, 1) * 8], I16,
                           kind="ExternalInput")
    spwd = nc.dram_tensor("spwd", [P, max(spc, 1)], F32,
                          kind="ExternalInput")
    spscd = nc.dram_tensor("spscd", [P, max(spc, 1) * 8], I16,
                           kind="ExternalInput")
    outd = nc.dram_tensor("out", [NPC, F_OUT], F32, kind="ExternalOutput")

    ag1_in = nc.dram_tensor("ag1_in", [NPC, F_HID], BF16, kind="Internal")
    ag1_outA = nc.dram_tensor("ag1_outA", [HALF_A, F_HID], BF16,
                              kind="Internal", addr_space="Shared")
    ag1_outB = nc.dram_tensor("ag1_outB", [HALF_B, F_HID], BF16,
                              kind="Internal", addr_space="Shared")
    s2tab = nc.dram_tensor("s2tab", [NPC, P], BF16, kind="Internal")
    rs_in = nc.dram_tensor("rs_in", [NTOT, F_OUT], F32, kind="Internal")
    rs_out = nc.dram_tensor("rs_out", [NPC, F_OUT], F32, kind="Internal")

    rg = [list(range(num_devices))]

    def ag(in_ap, out_ap):
        if with_ag:
            nc.gpsimd.collective_compute(
                "AllGather", mybir.AluOpType.bypass, replica_groups=rg,
                ins=[in_ap], outs=[out_ap])
        else:
            nc.sync.dma_start(out=out_ap[0:in_ap.shape[0], :], in_=in_ap)

    def rs(in_ap, out_ap):
        if with_ag:
            nc.gpsimd.collective_compute(
                "ReduceScatter", mybir.AluOpType.add, replica_groups=rg,
                ins=[in_ap], outs=[out_ap])
        else:
            nc.sync.dma_start(out=out_ap[:, :], in_=in_ap[0:NPC, :])

    with tile.TileContext(nc) as tc:
        with (
            tc.tile_pool(name="res", bufs=1) as rp,
            tc.tile_pool(name="mt", bufs=32) as mp,
        ):
            # ---------- resident constants ----------
            iota_i = rp.tile([P, P], mybir.dt.int32)
            nc.gpsimd.iota(iota_i[:], pattern=[[1, P]], base=0,
                           channel_multiplier=0)
            iota_bf = rp.tile([P, P], BF16)
            nc.vector.tensor_copy(iota_bf[:], iota_i[:])
            iotap_i = rp.tile([P, 1], mybir.dt.int32)
            nc.gpsimd.iota(iotap_i[:], pattern=[[0, 1]], base=0,
                           channel_multiplier=1)
            iotap_f = rp.tile([P, 1], F32)
            nc.vector.tensor_copy(iotap_f[:], iotap_i[:])
            ident = rp.tile([P, P], BF16)
            nc.vector.tensor_scalar(
                out=ident[:], in0=iota_bf[:], scalar1=iotap_f[:],
                scalar2=None, op0=mybir.AluOpType.is_equal)
            scr = rp.tile([P, 1], F32)
            for fn in (mybir.ActivationFunctionType.Lrelu,
                       mybir.ActivationFunctionType.Exp,
                       mybir.ActivationFunctionType.Ln,
                       mybir.ActivationFunctionType.Copy):
                nc.scalar.activation(scr[:], iota_bf[:, 0:1], fn, alpha=0.01)

            w2sb = rp.tile([P, 2, F_OUT], BF16)
            nc.sync.dma_start(
                out=w2sb[:], in_=W2p[:, :].rearrange("(k p) n -> p k n", p=P))
            b1sb = rp.tile([P, 2], F32)
            nc.sync.dma_start(out=b1sb[:], in_=b1t[:, :])
            b2sb = rp.tile([P, F_OUT], F32)
            nc.sync.dma_start(out=b2sb[:], in_=b2b[:, :])
            idxsb = rp.tile([P, idx_cols], I16)
            nc.sync.dma_start(out=idxsb[:], in_=idxd[:, :])
            dstsb = rp.tile([P, TILES * CH], F32)
            nc.sync.dma_start(out=dstsb[:], in_=dstd[:, :])
            wsb = rp.tile([P, TILES * CH], F32)
            nc.sync.dma_start(out=wsb[:], in_=wd[:, :])
            selsb = rp.tile([P, max(nch2, 1)], F32)
            nc.sync.dma_start(out=selsb[:], in_=seld[:, :])
            selwsb = rp.tile([P, max(nch2, 1)], F32)
            nc.sync.dma_start(out=selwsb[:], in_=selwd[:, :])
            scidxsb = rp.tile([P, max(sc_cols, 8)], I16)
            nc.sync.dma_start(out=scidxsb[:], in_=scidxd[:, :])
            spgisb = rp.tile([P, max(spc, 1) * 8], I16)
            nc.sync.dma_start(out=spgisb[:], in_=spgid[:, :])
            spwsb = rp.tile([P, max(spc, 1)], F32)
            nc.sync.dma_start(out=spwsb[:], in_=spwd[:, :])
            spscsb = rp.tile([P, max(spc, 1) * 8], I16)
            nc.sync.dma_start(out=spscsb[:], in_=spscd[:, :])

            acc = {}
            for h in range(2):
                for par in range(2):
                    a = rp.tile([P, NGRP, F_OUT], F32,
                                tag=f"acc{h}{par}", name=f"acc{h}{par}")
                    nc.gpsimd.memset(a[:], 0.0)
                    acc[(h, par)] = a[:]

            NCHL = TILES * CL
            NCHU = TILES * CU

            class Stream:
                def __init__(self, pool, tag, idx_sb, tab, nch, base_col,
                             esz, gbsz):
                    self.pool, self.tag = pool, tag
                    self.idx_sb, self.tab = idx_sb, tab
                    self.nch, self.base_col = nch, base_col
                    self.esz, self.gbsz = esz, gbsz
                    self.next = 0
                    self.bufs = {}

                def issue(self):
                    k = self.next
                    c0 = k * self.gbsz
                    n = min(self.gbsz, self.nch - c0)
                    gb = self.pool.tile([P, n, self.esz], BF16, tag=self.tag)
                    nc.gpsimd.dma_gather(
                        out_ap=gb[:], in_ap=self.tab,
                        idxs_ap=self.idx_sb[:, (self.base_col + c0) * 8:
                                            (self.base_col + c0 + n) * 8],
                        num_idxs=n * P, num_idxs_reg=n * P,
                        elem_size=self.esz)
                    self.bufs[k] = gb
                    self.next += 1

                def get(self, g):
                    while self.next * self.gbsz <= g:
                        self.issue()
                    return self.bufs[g // self.gbsz], g % self.gbsz

            with tc.tile_pool(name="gbL", bufs=14) as gLp:

                # ---------- phase 1: support = x @ W1 ----------
                with (
                    tc.tile_pool(name="p1w", bufs=1) as p1w,
                    tc.tile_pool(name="p1x", bufs=3) as p1x,
                    tc.tile_pool(name="p1ps", bufs=4, space="PSUM") as p1ps,
                ):
                    w1sb = p1w.tile([P, KT, F_HID], W1DT)
                    nc.sync.dma_start(
                        out=w1sb[:],
                        in_=W1p[:, :].rearrange("p (k n) -> p k n", k=KT))
                    blocks = []
                    b0 = 0
                    while b0 < NPC:
                        bsz = min(NB,
                                  (NPCA - b0) if b0 < NPCA else (NPC - b0))
                        blocks.append((b0, bsz))
                        b0 += bsz
                    for b0, bsz in blocks:
                        xsb = p1x.tile([P, KT, bsz], XDT, tag="xsb")
                        nc.sync.dma_start(
                            out=xsb[:],
                            in_=xT[:, b0:b0 + bsz].rearrange(
                                "(k p) n -> p k n", p=P))
                        nm = bsz // P
                        sup = p1x.tile([P, nm, F_HID], BF16, tag="sup")
                        for m in range(nm):
                            ps = p1ps.tile([P, F_HID], F32, tag="p1")
                            for k in range(KT // 2):
                                nc.tensor.matmul(
                                    ps[:],
                                    lhsT=xsb[:, 2 * k:2 * k + 2,
                                             m * P:(m + 1) * P],
                                    rhs=w1sb[:, 2 * k:2 * k + 2, :],
                                    start=(k == 0),
                                    stop=(k == KT // 2 - 1),
                                    perf_mode=mybir.MatmulPerfMode.DoubleRow)
                            nc.scalar.activation(
                                sup[:, m, :], ps[:],
                                mybir.ActivationFunctionType.Copy)
                        nc.sync.dma_start(
                            out=ag1_in[b0:b0 + bsz, :].rearrange(
                                "(m p) f -> p m f", p=P),
                            in_=sup[:])
                        if b0 + bsz == NPCA:
                            ag(ag1_in[0:NPCA, :], ag1_outA[:, :])
                    ag(ag1_in[NPCA:NPC, :], ag1_outB[:, :])

                with (
                    tc.tile_pool(name="big", bufs=1) as bigp,
                    tc.tile_pool(name="g1up", bufs=6) as g1up,
                    tc.tile_pool(name="smp", bufs=1) as smp,
                    tc.tile_pool(name="spgp", bufs=2) as spgp,
                    tc.tile_pool(name="s2p", bufs=12) as s2p,
                    tc.tile_pool(name="selp", bufs=2) as selp,
                    tc.tile_pool(name="ring", bufs=3) as ringp,
                    tc.tile_pool(name="ps", bufs=1, space="PSUM") as pp,
                    tc.tile_pool(name="ps2", bufs=3, space="PSUM") as pp2,
                    tc.tile_pool(name="ps2b", bufs=2, space="PSUM") as pp2b,
                    tc.tile_pool(name="ps3", bufs=1, space="PSUM") as pp3,
                ):
                    h1T = bigp.tile([P, 2, TILES * P], BF16)

                    g1L = Stream(gLp, "g1L", idxsb, ag1_outA[:, :], NCHL,
                                 0, F_HID, GB1)
                    g1U = Stream(g1up, "g1U", idxsb, ag1_outB[:, :], NCHU,
                                 NCHL, F_HID, GB1)

                    def get1(t, c):
                        if c < CL:
                            return g1L.get(t * CL + c)
                        return g1U.get(t * CU + (c - CL))

                    # ------ L2 scatter machinery ------
                    h_base = {0: 0, 1: nch_h[0]}
                    pend = {0: [], 1: []}   # (ci, s2sb) per open call
                    pst_call = {}
                    call_ptr = {0: 0, 1: 0}

                    selT_call = {}

                    def flush_call(h):
                        lo, hi = call_bounds[h][call_ptr[h]]
                        n = hi - lo
                        col0 = (h_base[h] + lo) * 8
                        selT = selT_call[h]
                        psm = pp2b.tile([P, SC_B, F_OUT], F32, tag="psm",
                                        name="psm")
                        for ci, s2sb in pend[h]:
                            nc.tensor.matmul(psm[:, ci, :],
                                             lhsT=selT[:, ci, :],
                                             rhs=s2sb[:],
                                             start=(ci == 0),
                                             stop=(ci == n - 1),
                                             skip_group_check=True)
                        ring = ringp.tile([P, SC_B, F_OUT], F32,
                                          tag=f"ring{h}", name=f"ring{h}")
                        nc.scalar.activation(
                            ring[:, 0:n, :], psm[:, 0:n, :],
                            mybir.ActivationFunctionType.Copy)
                        nc.gpsimd.dma_scatter_add(
                            acc[(h, 0)], ring[:, 0:n, :],
                            scidxsb[:, col0:col0 + n * 8],
                            n * P, n * P, F_OUT,
                            sbuf_tokens_per_rank=P, parity_reg=0,
                            out_ap_other=acc[(h, 1)])
                        call_ptr[h] += 1
                        pend[h] = []

                    def s2_tile(t):
                        ps5 = pp3.tile([P, F_OUT], F32, tag="ps5")
                        for hh in range(2):
                            nc.tensor.matmul(
                                ps5[:], lhsT=h1T[:, hh, t * P:(t + 1) * P],
                                rhs=w2sb[:, hh, :], start=(hh == 0),
                                stop=(hh == 1))
                        s2sb = s2p.tile([P, F_OUT], BF16, tag="s2")
                        nc.scalar.activation(
                            s2sb[:], ps5[:], mybir.ActivationFunctionType.Copy)
                        nc.sync.dma_start(
                            out=s2tab[t * P:(t + 1) * P, 0:F_OUT],
                            in_=s2sb[:])
                        return s2sb

                    def l2_chunk(gci, s2sb):
                        h = chunk_h[gci]
                        lo, hi = call_bounds[h][call_ptr[h]]
                        ci = chunk_seq[gci] - lo
                        s, cis = ci // SEG, ci % SEG
                        last = chunk_seq[gci] + 1 == hi
                        sel = mp.tile([P, P], BF16, tag="sel")
                        nc.vector.tensor_scalar(
                            out=sel[:], in0=iota_bf[:],
                            scalar1=selsb[:, gci:gci + 1],
                            scalar2=selwsb[:, gci:gci + 1],
                            op0=mybir.AluOpType.is_equal,
                            op1=mybir.AluOpType.mult)
                        if cis == 0:
                            pst_call[h] = pp2.tile([P, SEG, P], BF16,
                                                   tag="pst",
                                                   name=f"pst{h}")
                        nc.tensor.matmul(pst_call[h][:, cis, :], lhsT=sel[:],
                                         rhs=ident[:], is_transpose=True,
                                         start=(cis == 0),
                                         stop=(cis == SEG - 1 or last),
                                         skip_group_check=True)
                        pend[h].append((ci, s2sb))
                        if cis == SEG - 1 or last:
                            if s == 0:
                                selT_call[h] = selp.tile(
                                    [P, SC_B, P], BF16, tag=f"selT{h}",
                                    name=f"selT{h}")
                            if s % 2 == 0:
                                nc.vector.tensor_copy(
                                    selT_call[h][:,
                                                 s * SEG:s * SEG + cis + 1, :],
                                    pst_call[h][:, 0:cis + 1, :])
                            else:
                                nc.scalar.activation(
                                    selT_call[h][:,
                                                 s * SEG:s * SEG + cis + 1, :],
                                    pst_call[h][:, 0:cis + 1, :],
                                    mybir.ActivationFunctionType.Copy)
                        if last:
                            flush_call(h)

                    def l1_tile(t):
                        psT0 = pp.tile([P, P], F32, tag="psT0")
                        psT1 = pp.tile([P, P], F32, tag="psT1")
                        psT = [psT0, psT1]
                        for c in range(CH):
                            gb, slot = get1(t, c)
                            mt = mp.tile([P, P], BF16, tag="mt1")
                            nc.vector.tensor_scalar(
                                out=mt[:], in0=iota_bf[:],
                                scalar1=dstsb[:, t * CH + c:t * CH + c + 1],
                                scalar2=wsb[:, t * CH + c:t * CH + c + 1],
                                op0=mybir.AluOpType.is_equal,
                                op1=mybir.AluOpType.mult)
                            for hh in range(2):
                                nc.tensor.matmul(
                                    psT[hh][:],
                                    lhsT=gb[:, slot, hh * P:(hh + 1) * P],
                                    rhs=mt[:],
                                    start=(c == 0), stop=(c == CH - 1))
                        for hh in range(2):
                            nc.scalar.activation(
                                h1T[:, hh, t * P:(t + 1) * P], psT[hh][:],
                                mybir.ActivationFunctionType.Lrelu,
                                bias=b1sb[:, hh:hh + 1],
                                scale=(1.0 / W1_SCALE) if DR else 1.0,
                                alpha=0.01)

                    tile_chunks = {}
                    for gci, t in enumerate(chunk_tile):
                        tile_chunks.setdefault(t, []).append(gci)

                    def spill_call(lo, hi, h, bound):
                        n = hi - lo
                        gb = spgp.tile([P, n, P], BF16, tag="spg",
                                       name="spg")
                        nc.gpsimd.dma_gather(
                            out_ap=gb[:], in_ap=s2tab[0:(bound + 1) * P, :],
                            idxs_ap=spgisb[:, lo * 8:hi * 8],
                            num_idxs=n * P, num_idxs_reg=n * P,
                            elem_size=P)
                        rbuf = ringp.tile([P, n, F_OUT], F32, tag="spr",
                                          name="spr")
                        for c in range(n):
                            nc.vector.tensor_scalar(
                                out=rbuf[:, c, :],
                                in0=gb[:, c, 0:F_OUT],
                                scalar1=spwsb[:, lo + c:lo + c + 1],
                                scalar2=None,
                                op0=mybir.AluOpType.mult)
                        nc.gpsimd.dma_scatter_add(
                            acc[(h, 0)], rbuf[:, 0:n, :],
                            spscsb[:, lo * 8:hi * 8],
                            n * P, n * P, F_OUT,
                            sbuf_tokens_per_rank=P, parity_reg=0,
                            out_ap_other=acc[(h, 1)])

                    sp_by_tile = {}
                    for (lo, hi, h, mt_) in sp_calls:
                        sp_by_tile.setdefault(min(mt_, TILES - 1),
                                              []).append((lo, hi, h, mt_))

                    for _ in range(14):
                        if g1L.next * g1L.gbsz < g1L.nch:
                            g1L.issue()
                    for t in range(TILES):
                        l1_tile(t)
                        s2sb = s2_tile(t)
                        for gci in tile_chunks.get(t, []):
                            l2_chunk(gci, s2sb)
                        for (lo, hi, h, mt_) in sp_by_tile.get(t, []):
                            spill_call(lo, hi, h, mt_)
                    for h in range(2):
                        if pend[h]:
                            flush_call(h)
                    for fn in (mybir.ActivationFunctionType.Exp,
                               mybir.ActivationFunctionType.Ln):
                        nc.scalar.activation(scr[:], iota_bf[:, 0:1], fn)

                    # ------ dump accumulators + ReduceScatter ------
                    for h in range(2):
                        for par in range(2):
                            base = h * 4 * NPC + par * 2 * NPC
                            nc.sync.dma_start(
                                out=rs_in[base:base + 2 * NPC, :].rearrange(
                                    "(p g) f -> p g f", p=P),
                                in_=acc[(h, par)][:, 0:GHALF, :])
                    rs(rs_in[:, :], rs_out[:, :])

                    # ------ phase 7: bias + log_softmax + out ------
                    zsb = bigp.tile([P, TILES, F_OUT], F32)
                    nc.sync.dma_start(
                        out=zsb[:],
                        in_=rs_out[:, :].rearrange("(p q) f -> p q f", p=P))
                    zb = smp.tile([P, TILES, F_OUT], F32, tag="zb")
                    nc.vector.tensor_tensor(
                        out=zb[:], in0=zsb[:],
                        in1=b2sb[:][:, None, :].to_broadcast(
                            [P, TILES, F_OUT]),
                        op=mybir.AluOpType.add)
                    mx = smp.tile([P, TILES], F32, tag="mx")
                    nc.vector.tensor_reduce(out=mx[:], in_=zb[:],
                                            op=mybir.AluOpType.max,
                                            axis=mybir.AxisListType.X)
                    tsub = smp.tile([P, TILES, F_OUT], F32, tag="tsub")
                    nc.vector.tensor_tensor(
                        out=tsub[:], in0=zb[:],
                        in1=mx[:][:, :, None].to_broadcast([P, TILES, F_OUT]),
                        op=mybir.AluOpType.subtract)
                    ex = smp.tile([P, TILES, F_OUT], F32, tag="ex")
                    nc.scalar.activation(ex[:], tsub[:],
                                         mybir.ActivationFunctionType.Exp)
                    sm = smp.tile([P, TILES], F32, tag="sm")
                    nc.vector.tensor_reduce(out=sm[:], in_=ex[:],
                                            op=mybir.AluOpType.add,
                                            axis=mybir.AxisListType.X)
                    ls = smp.tile([P, TILES], F32, tag="ls")
                    nc.scalar.activation(ls[:], sm[:],
                                         mybir.ActivationFunctionType.Ln)
                    res = smp.tile([P, TILES, F_OUT], F32, tag="res")
                    nc.vector.tensor_tensor(
                        out=res[:], in0=tsub[:],
                        in1=ls[:][:, :, None].to_broadcast([P, TILES, F_OUT]),
                        op=mybir.AluOpType.subtract)
                    nc.sync.dma_start(
                        out=outd[:, :].rearrange("(p q) f -> p q f", p=P),
                        in_=res[:])

    nc.compile()
    return nc


def _preprocess(x, edge_src, edge_dst, edge_weight, W1, b1, W2, b2):
    import ml_dtypes
    bf16 = ml_dtypes.bfloat16
    xdt = ml_dtypes.float8_e4m3 if X_FP8 else bf16

    x = np.asarray(x, dtype=np.float32)
    edge_src = np.asarray(edge_src, dtype=np.int64)
    edge_dst = np.asarray(edge_dst, dtype=np.int64)
    edge_weight = np.asarray(edge_weight, dtype=np.float32)
    W1 = np.asarray(W1, dtype=np.float32)
    b1 = np.asarray(b1, dtype=np.float32)
    W2 = np.asarray(W2, dtype=np.float32)
    b2 = np.asarray(b2, dtype=np.float32)

    NBINS = CORES * TILES
    deg = np.bincount(edge_dst, minlength=N_NODES)

    import heapq
    order = np.argsort(-deg, kind="stable")
    heap = [(0, b) for b in range(NBINS)]
    heapq.heapify(heap)
    counts = np.zeros(NBINS, dtype=np.int64)
    node_row = np.empty(N_NODES, dtype=np.int64)
    for nid in order:
        while True:
            load, b = heapq.heappop(heap)
            if counts[b] < P:
                break
        core, t = b // TILES, b % TILES
        node_row[nid] = core * NPC + t * P + counts[b]
        counts[b] += 1
        if counts[b] < P:
            heapq.heappush(heap, (load + int(deg[nid]), b))

    src_row = node_row[edge_src]
    dst_row = node_row[edge_dst]

    # ---------------- L1 (dst-grouped gather), unchanged ----------------
    core_e = dst_row // NPC
    t_e = (dst_row % NPC) // P
    lane_d = dst_row % P
    src_core = src_row // NPC
    src_local = src_row % NPC
    half_e = (src_local >= NPCA).astype(np.int64)
    loc_src = np.where(half_e == 0, src_core * NPCA + src_local,
                       src_core * NPCB + (src_local - NPCA))

    key = (core_e * TILES + t_e) * 2 + half_e
    sort_i = np.argsort(key, kind="stable")
    ks = key[sort_i]
    cnt = np.bincount(ks, minlength=NBINS * 2)
    starts = np.zeros(NBINS * 2, dtype=np.int64)
    starts[1:] = np.cumsum(cnt)[:-1]
    pos_sorted = np.arange(N_EDGES) - starts[ks]
    pos = np.empty(N_EDGES, dtype=np.int64)
    pos[sort_i] = pos_sorted

    nL = cnt[0::2].reshape(CORES, TILES)
    nU = cnt[1::2].reshape(CORES, TILES)
    CL = max(1, int(np.ceil(nL.max() / P)))
    CU = max(1, int(np.ceil(nU.max() / P)))
    CH = CL + CU
    idx_cols = TILES * CH * 8

    g_stream = np.where(half_e == 0, t_e * CL + pos // P,
                        t_e * CU + pos // P)
    sbase = np.where(half_e == 0, 0, TILES * CL * 8)
    lane_s = pos % P
    idx_col = sbase + g_stream * 8 + lane_s // 16
    idx_par = lane_s % 16

    idx_arr = np.zeros((CORES, 16, idx_cols), dtype=np.int16)
    idx_arr[core_e, idx_par, idx_col] = loc_src.astype(np.int16)

    c_e = np.where(half_e == 0, pos // P, CL + pos // P)
    lane_e = pos % P
    dcol = t_e * CH + c_e
    dst_arr = np.zeros((CORES, P, TILES * CH), dtype=np.float32)
    w_arr = np.zeros((CORES, P, TILES * CH), dtype=np.float32)
    dst_arr[core_e, lane_e, dcol] = lane_d.astype(np.float32)
    w_arr[core_e, lane_e, dcol] = edge_weight

    # ---------------- L2 (src-grouped scatter) ----------------
    dc = dst_row // NPC
    di = dst_row % NPC
    d_h = (dc // 4).astype(np.int64)
    d_slot = 2 * (di % GHALF) + (dc % 4) // 2
    d_p = 64 * (dc % 2) + di // GHALF
    d_tok = (d_slot * P + d_p).astype(np.int64)

    s_tile = (src_local // P).astype(np.int64)
    s_lane = (src_local % P).astype(np.int64)

    # per-core per-(tile, h) edge lists
    cnt2 = np.zeros((CORES, TILES, 2), dtype=np.int64)
    np.add.at(cnt2, (src_core, s_tile, d_h), 1)
    nch_th = np.maximum(0, np.ceil(cnt2.max(axis=0) / P)).astype(np.int64)

    # canonical chunk sequence: t asc, h=0 chunks then h=1 chunks
    chunk_tile = []
    chunk_h = []
    chunk_seq = []
    chunk_of = {}       # (t, h, k) -> gci
    nch_h = {0: 0, 1: 0}
    for t in range(TILES):
        for h in range(2):
            for k in range(int(nch_th[t, h])):
                chunk_of[(t, h, k)] = len(chunk_tile)
                chunk_tile.append(t)
                chunk_h.append(h)
                chunk_seq.append(nch_h[h])
                nch_h[h] += 1
    nch2 = len(chunk_tile)
    call_bounds = {0: [], 1: []}
    for h in range(2):
        lo = 0
        while lo < nch_h[h]:
            hi = min(lo + SC_B, nch_h[h])
            call_bounds[h].append((lo, hi))
            lo = hi
    # call id per per-h seq number
    call_of_seq = {0: {}, 1: {}}
    for h in range(2):
        for cid, (lo, hi) in enumerate(call_bounds[h]):
            for s in range(lo, hi):
                call_of_seq[h][s] = cid

    # per-core fill with ejection
    sel_src = np.full((CORES, P, max(nch2, 1)), -1.0, dtype=np.float32)
    sel_w = np.zeros((CORES, P, max(nch2, 1)), dtype=np.float32)
    sc_tok = np.zeros((CORES, P, max(nch2, 1)), dtype=np.int64)
    sc_tok[:, :, :] = DUMP0 + np.arange(P)[None, :, None]
    spills = [[[], []] for _ in range(CORES)]   # [core][h] -> list

    eorder = np.lexsort((d_h * 0, d_h, s_tile, src_core))
    es_core = src_core[eorder]
    es_tile = s_tile[eorder]
    es_h = d_h[eorder]
    es_lane = s_lane[eorder]
    es_w = edge_weight[eorder]
    es_tok = d_tok[eorder]
    es_srcloc = src_local[eorder]

    fill_k = {}
    seen = {}
    for j in range(len(es_core)):
        c = int(es_core[j]); t = int(es_tile[j]); h = int(es_h[j])
        tok = int(es_tok[j])
        kk = fill_k.get((c, t, h), 0)
        ci = kk // P
        lane = kk % P
        gci = chunk_of[(t, h, ci)]
        sq = chunk_seq[gci]
        cid = call_of_seq[h][sq]
        skey = (c, h, cid)
        sset = seen.setdefault(skey, set())
        if tok in sset:
            spills[c][h].append((int(es_srcloc[j]), float(es_w[j]), tok))
        else:
            sset.add(tok)
            sel_src[c, lane, gci] = float(es_lane[j])
            sel_w[c, lane, gci] = float(es_w[j])
            sc_tok[c, lane, gci] = tok
        fill_k[(c, t, h)] = kk + 1

    # spill scheduling: per core/h, greedy calls (sorted by src tile) with
    # unique tokens per call, <= SP_CAP chunks per call
    sp_calls_core = [[[], []] for _ in range(CORES)]  # [core][h] -> calls
    for c in range(CORES):
        for h in range(2):
            items = sorted(spills[c][h], key=lambda it: it[0] // P)
            calls, seens = [], []
            for it in items:
                placed = False
                for ci in range(len(calls)):
                    if it[2] not in seens[ci] and len(calls[ci]) < SP_CAP * P:
                        calls[ci].append(it)
                        seens[ci].add(it[2])
                        placed = True
                        break
                if not placed:
                    calls.append([it])
                    seens.append({it[2]})
            sp_calls_core[c][h] = calls
    # canonical: per h, ncalls = max; per call, chunks = max, maxtile = max
    ncalls_h = {h: max(len(sp_calls_core[c][h]) for c in range(CORES))
                for h in range(2)}
    sp_calls = []       # (chunk_lo, chunk_hi, h, maxtile)
    chunk_cursor = 0
    call_layout = []    # (h, k, chunk_lo)
    for h in range(2):
        for k in range(ncalls_h[h]):
            nchk = 1
            mt_ = 0
            for c in range(CORES):
                calls = sp_calls_core[c][h]
                if k < len(calls) and calls[k]:
                    nchk = max(nchk, (len(calls[k]) + P - 1) // P)
                    mt_ = max(mt_, max(it[0] // P for it in calls[k]))
            sp_calls.append((chunk_cursor, chunk_cursor + nchk, h, mt_))
            call_layout.append((h, k, chunk_cursor))
            chunk_cursor += nchk
    spc = chunk_cursor

    sp_gi = np.zeros((CORES, P, max(spc, 1)), dtype=np.int64)
    sp_w = np.zeros((CORES, P, max(spc, 1)), dtype=np.float32)
    sp_sc = np.zeros((CORES, P, max(spc, 1)), dtype=np.int64)
    sp_sc[:, :, :] = DUMP0 + np.arange(P)[None, :, None]
    for (h, k, lo) in call_layout:
        for c in range(CORES):
            calls = sp_calls_core[c][h]
            if k >= len(calls):
                continue
            for i, (srcloc, w, tok) in enumerate(calls[k]):
                sp_gi[c, i % P, lo + i // P] = srcloc
                sp_w[c, i % P, lo + i // P] = w
                sp_sc[c, i % P, lo + i // P] = tok

    def wrap16(tok3):
        # [CORES, P, ncc] -> [CORES, 128, ncc*8] int16 (idx i of chunk cc at
        # partition i%16, col cc*8 + i//16; replicated x8 vertically)
        ncc = tok3.shape[2]
        out = np.zeros((CORES, 16, ncc * 8), dtype=np.int16)
        i = np.arange(P)
        for cc in range(ncc):
            out[:, i % 16, cc * 8 + i // 16] = tok3[:, i, cc].astype(np.int16)
        return np.tile(out, (1, 8, 1))

    # scatter idx table columns ordered by (h, per-h seq)
    sc_tok_h = np.zeros((CORES, P, max(nch2, 1)), dtype=np.int64)
    sc_tok_h[:, :, :] = DUMP0 + np.arange(P)[None, :, None]
    for gci in range(nch2):
        h = chunk_h[gci]
        col = (0 if h == 0 else nch_h[0]) + chunk_seq[gci]
        sc_tok_h[:, :, col] = sc_tok[:, :, gci]
    scidx_arr = wrap16(sc_tok_h)
    spgi_arr = wrap16(sp_gi)
    spsc_arr = wrap16(sp_sc)

    sched = dict(chunk_tile=chunk_tile, chunk_h=chunk_h,
                 chunk_seq=chunk_seq, call_bounds=call_bounds,
                 nch_h=nch_h, sp_calls=sp_calls, spc=spc)

    # ---------------- dense inputs ----------------
    W1p = np.zeros((KPAD, F_HID), dtype=np.float32)
    W1p[:F_IN] = W1
    if DR:
        w1dt = ml_dtypes.float8_e4m3
        W1p_c = (W1p * W1_SCALE).astype(w1dt)
    else:
        W1p_c = W1p.astype(bf16)
    # p-major layout: partition p holds its KT k-rows contiguously so the
    # load uses one >=512B descriptor per partition (256B rows pay the 2x
    # sub-512B DMA penalty otherwise)
    W1p_c = np.ascontiguousarray(
        W1p_c.reshape(KT, P, F_HID).transpose(1, 0, 2).reshape(
            P, KT * F_HID))
    b1t = np.ascontiguousarray(b1.reshape(2, P).T.astype(np.float32))
    b2b = np.broadcast_to(b2, (P, F_OUT)).copy()

    in_maps = []
    row_node = np.full(NTOT, -1, dtype=np.int64)
    row_node[node_row] = np.arange(N_NODES)
    for c in range(CORES):
        rows = row_node[c * NPC:(c + 1) * NPC]
        xc = np.zeros((NPC, F_IN), dtype=np.float32)
        occ = rows >= 0
        xc[occ] = x[rows[occ]]
        xTc = np.zeros((KPAD, NPC), dtype=xdt)
        xTc[:F_IN] = xc.T.astype(xdt)
        in_maps.append(dict(
            xT=xTc,
            W1p=W1p_c,
            W2p=np.ascontiguousarray(W2.astype(bf16)),
            b1t=b1t,
            b2b=b2b,
            idxd=np.tile(idx_arr[c], (8, 1)),
            dstd=dst_arr[c],
            wd=w_arr[c],
            seld=np.ascontiguousarray(sel_src[c]),
            selwd=np.ascontiguousarray(sel_w[c]),
            scidxd=np.ascontiguousarray(scidx_arr[c]),
            spgid=np.ascontiguousarray(spgi_arr[c]),
            spwd=np.ascontiguousarray(sp_w[c]),
            spscd=np.ascontiguousarray(spsc_arr[c]),
        ))
    return in_maps, node_row, CL, CU, sched


def _sched_key(CL, CU, sched):
    return (CL, CU, tuple(sched["chunk_tile"]), tuple(sched["chunk_h"]),
            tuple(sched["sp_calls"]))


def kernel(**inputs):
    global LAST_RESULT
    in_maps, node_row, CL, CU, sched = _preprocess(**inputs)
    key = _sched_key(CL, CU, sched)
    if key not in _CACHE:
        _CACHE[key] = _build(CL, CU, sched)
    nc = _CACHE[key]
    res = run_bass_kernel_spmd(nc, in_maps, core_ids=list(range(CORES)))
    LAST_RESULT = res
    allout = np.concatenate([res.results[c]["out"] for c in range(CORES)],
                            axis=0)
    return np.ascontiguousarray(allout[node_row]).astype(np.float32)
